# revision 2
# baseline (speedup 1.0000x reference)
# LSTM (embedding -> single-layer LSTM -> linear head) on Trainium2, v2.
#
# Sharding: data-parallel over batch, B=64 -> 2 cores x 32.
#
# Per core: all-fp16 datapath. Gates live in PSUM as [128 part, 256 free]
# per H-half: partition groups {0-31: i, 32-63: f, 64-95: o, 96-127: g}
# (gate order permuted on host). tanh(g) is computed as 2*sigmoid(2g)-1
# with the 2x folded into the g-gate weights on host, so ONE sigmoid
# activation covers all 4 gates. The 2g'-1 fixup is a DVE tensor_scalar.
# Cross-partition placement (ACT out-shift, mixed-base TT) avoids all
# rebase matmuls: g' -> 0:32, c lives at 32:64, tanh(c) -> 64:96.
# H is split into 2 halves (N=256) pipelined through the engines; the
# next step's hh matmuls start as soon as the first half's hT is ready.
import numpy as np

_VOCAB, _EMB, _HID = 50257, 512, 512
_B, _T = 64, 1024
_NCORES = 2
_BLOC = 32
_HH = 256  # half of H


def _build(T, vocab, bloc=32, mode="full"):
    """Build the per-core Bass program. Returns (nc, input_names, out_name)."""
    import concourse.bass as bass
    import concourse.mybir as mybir
    import concourse.tile as tile
    from concourse import bacc
    from concourse.masks import make_identity

    f32 = mybir.dt.float32
    bf16 = mybir.dt.float16  # fp16 datapath (10-bit mantissa)
    i16 = mybir.dt.int16
    SIG = mybir.ActivationFunctionType.Sigmoid
    TANH = mybir.ActivationFunctionType.Tanh
    MUL = mybir.AluOpType.mult
    ADD = mybir.AluOpType.add
    SUB = mybir.AluOpType.subtract

    TPC = 128 // bloc            # timesteps per 128-token chunk (4)
    assert T % TPC == 0
    NCH = T // TPC               # number of chunks
    H4 = 4 * _HID

    nc = bacc.Bacc("TRN2", target_bir_lowering=False, debug=False,
                   num_swdge_queues=4)

    # ---- DRAM I/O (per-core) ----
    embc_d = nc.dram_tensor("embc", [32768, _EMB], bf16, kind="ExternalInput")
    xidx_d = nc.dram_tensor("xidx16", [128, NCH, 8], i16, kind="ExternalInput")
    whhT_d = nc.dram_tensor("whhT", [128, 4, H4], bf16, kind="ExternalInput")
    wihT_d = nc.dram_tensor("wihT", [128, 4, H4], bf16, kind="ExternalInput")
    bias_d = nc.dram_tensor("bias", [1, H4], bf16, kind="ExternalInput")
    woutT_d = nc.dram_tensor("woutT", [128, 4, 2], bf16, kind="ExternalInput")
    bout_d = nc.dram_tensor("bout", [1, 2], bf16, kind="ExternalInput")
    out_d = nc.dram_tensor("out", [bloc, 2], f32, kind="ExternalOutput")

    with tile.TileContext(nc) as tc:
        with (
            tc.tile_pool(name="const", bufs=1) as pc,
            tc.tile_pool(name="state", bufs=1) as ps,
            tc.tile_pool(name="eT", bufs=4) as peT,
            tc.tile_pool(name="xb", bufs=2) as pxb,
            tc.tile_pool(name="act", bufs=2) as pact,
            tc.tile_pool(name="dve", bufs=2) as pdv,
            tc.tile_pool(name="hT", bufs=2) as phT,
            tc.tile_pool(name="ps_g", bufs=2, space="PSUM") as pp_g,
            tc.tile_pool(name="ps_t", bufs=2, space="PSUM") as pp_t,
            tc.tile_pool(name="ps_x", bufs=2, space="PSUM") as pp_x,
        ):
            # ---- weights ----
            whhT = pc.tile([128, 4, H4], bf16)
            nc.sync.dma_start(whhT[:], whhT_d.ap())
            wihT = pc.tile([128, 4, H4], bf16)
            nc.sync.dma_start(wihT[:], wihT_d.ap())
            bias_t = pc.tile([1, H4], bf16)
            nc.sync.dma_start(bias_t[:], bias_d.ap())
            woutT = pc.tile([128, 4, 2], bf16)
            nc.sync.dma_start(woutT[:], woutT_d.ap())
            bout_t = pc.tile([1, 2], bf16)
            nc.sync.dma_start(bout_t[:], bout_d.ap())

            from concourse import library_config
            nc.gpsimd.load_library(library_config.mlp)
            identb = pc.tile([128, 128], bf16)    # fp16 injects/transposes
            make_identity(nc, identb[:])
            idx_all = pc.tile([128, NCH, 8], i16)
            nc.sync.dma_start(idx_all[:], xidx_d.ap())
            ones_b = pc.tile([1, 128], bf16)
            nc.vector.memset(ones_b[:], 1.0)

            # persistent cell state, half h at [32:64, h, :], bf16
            c_sb = ps.tile([64, 2, _HH], bf16)
            nc.vector.memset(c_sb[32:64, :, :], 0.0)
            if mode in ("mm", "both", "noht"):
                h_const = ps.tile([96, _HH], bf16)
                nc.vector.memset(h_const[64:96, :], 0.01)
            if mode in ("chain", "both", "nosig"):
                psg_const = ps.tile([128, _HH], bf16)
                nc.vector.memset(psg_const[:], 0.01)

            # ---- x_proj producer ----
            def prod_gather(cb):
                # gathered+transposed: eT[p, k, i] = embc[idx_i, 128k + p]
                eT = peT.tile([128, 4, 128], bf16, tag="eT")
                nc.gpsimd.dma_gather(
                    eT[:], embc_d.ap(), idx_all[:, cb, :],
                    num_idxs=128, num_idxs_reg=128, elem_size=_EMB,
                    transpose=True, queue_num=cb % 4,
                )
                return eT

            def prod_mm(eT, j):
                # x_proj GEMM for gate j of one chunk + bias
                ps_x = pp_x.tile([128, 512], f32, space="PSUM", tag="x")
                for k in range(4):
                    nc.tensor.matmul(
                        ps_x[:], eT[:, k, :], wihT[:, k, j * 512:(j + 1) * 512],
                        start=(k == 0), stop=False,
                    )
                nc.tensor.matmul(
                    ps_x[:], ones_b[0:1, 0:128], bias_t[0:1, j * 512:(j + 1) * 512],
                    start=False, stop=True,
                )
                return ps_x

            def prod_copy(ps_x, xb, j):
                nc.scalar.copy(xb[:, j, :], ps_x[:])

            # ---- recurrence pieces ----
            def inject(t, xb, last):
                # start the gate accumulation for step t with x_proj (+bias)
                r = t % TPC
                rp = r * bloc
                g = pp_g.tile([128, 2, _HH], f32, space="PSUM", tag="g")
                for j in range(4):
                    nc.tensor.matmul(
                        g[j * 32:(j + 1) * 32, :, :],
                        identb[rp:rp + 32, rp:rp + 32],
                        xb[rp:rp + 32, j, :],
                        start=True, stop=last,
                        tile_position=(rp, j * 32),
                        skip_group_check=True,
                    )
                return g

            def hh_mm(psg, hT):
                for k in range(4):
                    for j in range(4):
                        nc.tensor.matmul(
                            psg[j * 32:(j + 1) * 32, :, :],
                            hT[:, k, :],
                            whhT[:, k, j * 512:(j + 1) * 512],
                            start=False, stop=(k == 3),
                            tile_position=(0, j * 32),
                            skip_group_check=True,
                        )

            def chain(t, psg):
                if mode == "mm":
                    return [h_const, h_const]
                if mode in ("chain", "both", "nosig"):
                    halves = [psg_const[:], psg_const[:]]
                else:
                    halves = [psg[:, 0, :], psg[:, 1, :]]
                # ACT+DVE per-half chains; returns h tiles
                sfo = [None, None]
                for h in (0, 1):
                    s = pact.tile([128, _HH], bf16, tag=f"sfo{h}")
                    nc.scalar.activation(s[:], halves[h], SIG)
                    sfo[h] = s
                figs = [None, None]
                for h in (0, 1):
                    g2 = pdv.tile([32, _HH], bf16, tag=f"g2{h}")
                    nc.vector.tensor_scalar(g2[:], sfo[h][96:128, :], 2.0, 1.0,
                                            MUL, SUB)
                    fc = pdv.tile([64, _HH], bf16, tag=f"fc{h}")
                    nc.vector.tensor_tensor(fc[32:64, :], sfo[h][32:64, :],
                                            c_sb[32:64, h, :], MUL)
                    ig = pdv.tile([64, _HH], bf16, tag=f"ig{h}")
                    nc.vector.tensor_tensor(ig[32:64, :], sfo[h][0:32, :],
                                            g2[:], MUL)
                    figs[h] = (fc, ig)
                for h in (1, 0):   # critical half-1 path first
                    fc, ig = figs[h]
                    nc.vector.tensor_tensor(c_sb[32:64, h, :], fc[32:64, :],
                                            ig[32:64, :], ADD)
                thcs = [None, None]
                for h in (1, 0):
                    thc = pact.tile([96, _HH], bf16, tag=f"thc{h}")
                    nc.scalar.activation(thc[64:96, :], c_sb[32:64, h, :], TANH)
                    thcs[h] = thc
                hs = [None, None]
                for h in (1, 0):
                    hh = pdv.tile([96, _HH], bf16, tag=f"h{h}")
                    nc.vector.tensor_tensor(hh[64:96, :], sfo[h][64:96, :],
                                            thcs[h][64:96, :], MUL)
                    hs[h] = hh
                return hs

            def transp(hs):
                if mode in ("both", "noht"):
                    hs = [h_const, h_const]
                hT = phT.tile([128, 4, 32], bf16)
                for h in (0, 1):
                    pst = pp_t.tile([128, 2, 32], bf16, space="PSUM",
                                    tag=f"t{h}")
                    for kk in range(2):
                        nc.tensor.transpose(
                            pst[:, kk, :],
                            hs[h][64:96, kk * 128:(kk + 1) * 128],
                            identb[64:96, 64:96],
                        )
                    nc.vector.tensor_copy(hT[:, 2 * h:2 * h + 2, :], pst[:])
                return hT

            # ---- main program ----
            # producer pipeline state: e_t gathered 2 chunks ahead; slices
            # of chunk cb+1 (transp/eT at r=0, x_proj MMs at r=1..3) run
            # during chunk cb so the PE always has filler work.
            if mode != "chain":
                eTs = {cb: prod_gather(cb) for cb in range(min(4, NCH))}
                xb = pxb.tile([128, 4, 512], bf16, tag="xb")
                for j in range(4):
                    prod_copy(prod_mm(eTs[0], j), xb, j)
                psg = inject(0, xb, last=True)
            else:
                xb = None
                psg = None
            hT = None
            for cb in range(NCH):
                for r in range(TPC):
                    t = cb * TPC + r
                    if hT is not None and mode != "chain":
                        hh_mm(psg, hT)
                    hs = chain(t, psg)
                    # producer MM slice for chunk cb+1 (+ gather for cb+4)
                    ps_x_r = None
                    if mode != "chain" and cb + 1 < NCH:
                        if r == 0:
                            if cb + 4 < NCH:
                                eTs[cb + 4] = prod_gather(cb + 4)
                            xb_n = pxb.tile([128, 4, 512], bf16, tag="xb")
                        with tc.high_priority(offset=-1_000_000):
                            ps_x_r = prod_mm(eTs[cb + 1], r)
                        if r == 3:
                            eTs.pop(cb, None)
                    if ps_x_r is not None:
                        with tc.high_priority(offset=-1_000_000):
                            prod_copy(ps_x_r, xb_n, r)
                    # inject next step (ready early; fills PE before transposes)
                    if t + 1 < T and mode != "chain":
                        xb_t = xb if r + 1 < TPC else xb_n
                        psg = inject(t + 1, xb_t, last=False)
                    hT = transp(hs)
                if mode != "chain" and cb + 1 < NCH:
                    xb = xb_n

            # ---- output head: out = h_last @ w_out.T + b_out ----
            ps_o = pp_t.tile([32, 2], f32, space="PSUM", tag="t0")
            for k in range(4):
                nc.tensor.matmul(
                    ps_o[:], hT[:, k, :], woutT[:, k, :],
                    start=(k == 0), stop=False,
                )
            nc.tensor.matmul(
                ps_o[:], ones_b[0:1, 0:32], bout_t[0:1, :],
                start=False, stop=True,
            )
            o_sb = pc.tile([32, 2], f32)
            nc.vector.tensor_copy(o_sb[:], ps_o[:])
            nc.sync.dma_start(out_d.ap(), o_sb[:])

    nc.compile()
    in_names = ["embc", "xidx16", "whhT", "wihT", "bias", "woutT", "bout"]
    return nc, in_names, "out"


def _prep_host(x, emb, w_ih, w_hh, b_ih, b_hh, w_out, b_out, bloc, ncores):
    """Host-side reshapes: gate permutation [i,f,g,o] -> [i,f,o,g], g-gate
    rows scaled by 2 (tanh via sigmoid), weight transposes into
    [128, 4, *] K-major tiles, per-core t-major index lists."""
    bf16 = np.float16
    H = _HID

    def perm_rows(w):
        # [i, f, g, o] -> [i, f, o, g] with g scaled by 2
        return np.concatenate(
            [w[0:H], w[H:2 * H], w[3 * H:4 * H], 2.0 * w[2 * H:3 * H]], axis=0)

    w_ih_p = perm_rows(np.asarray(w_ih, np.float32))
    w_hh_p = perm_rows(np.asarray(w_hh, np.float32))
    bias_p = perm_rows((np.asarray(b_ih, np.float32)
                        + np.asarray(b_hh, np.float32))[:, None])[:, 0]

    # wT[p, k, n] = w_p[n, 128k + p]
    def to_kt(w_p):
        return np.ascontiguousarray(
            w_p.T.reshape(4, 128, w_p.shape[0]).transpose(1, 0, 2))

    whhT = to_kt(w_hh_p).astype(bf16)
    wihT = to_kt(w_ih_p).astype(bf16)
    woutT = np.ascontiguousarray(
        np.asarray(w_out, np.float32).T.reshape(4, 128, 2).transpose(1, 0, 2)
    ).astype(bf16)

    emb16 = np.asarray(emb, np.float32).astype(bf16)
    bias_c = np.ascontiguousarray(bias_p[None, :]).astype(bf16)
    bout_c = np.ascontiguousarray(np.asarray(b_out, np.float32)[None, :]).astype(bf16)

    x = np.asarray(x)
    B, T = x.shape
    NCH = T // (128 // bloc)
    in_maps = []
    for c in range(ncores):
        xs = x[c * bloc:(c + 1) * bloc, :]          # [bloc, T]
        # per-core compact table: <=bloc*T distinct ids -> int16 indices
        uniq, inv = np.unique(xs, return_inverse=True)
        assert len(uniq) <= 32768
        embc = np.zeros((32768, _EMB), bf16)
        embc[:len(uniq)] = emb16[uniq]
        # arr[cb, 32r+b] = inv[b, 4cb+r]
        arr = inv.reshape(bloc, NCH, 4).transpose(1, 2, 0).reshape(NCH, 128)
        # wrap: idx i of chunk cb at [i%16, cb, i//16], replicated into
        # all 8 GPSIMD-core partition groups
        idx16 = np.tile(arr.reshape(NCH, 8, 16).transpose(2, 0, 1).astype(np.int16),
                        (8, 1, 1))
        in_maps.append({
            "embc": embc,
            "xidx16": np.ascontiguousarray(idx16),
            "whhT": whhT,
            "wihT": wihT,
            "bias": bias_c,
            "woutT": woutT,
            "bout": bout_c,
        })
    return in_maps


_CACHE = {}


def kernel(x, emb, w_ih, w_hh, b_ih, b_hh, w_out, b_out):
    from concourse.bass_utils import run_bass_kernel_spmd

    x = np.asarray(x)
    B, T = x.shape
    ncores = _NCORES
    bloc = B // ncores
    vocab = emb.shape[0]

    key = (T, vocab, bloc)
    if key not in _CACHE:
        _CACHE[key] = _build(T, vocab, bloc)
    nc, in_names, out_name = _CACHE[key]

    in_maps = _prep_host(x, emb, w_ih, w_hh, b_ih, b_hh, w_out, b_out, bloc, ncores)
    res = run_bass_kernel_spmd(nc, in_maps, core_ids=list(range(ncores)))
    out = np.concatenate([r[out_name] for r in res.results], axis=0)  # [B, 2]
    return out


if __name__ == "__main__":
    _build(_T, _VOCAB, _BLOC)
    print("build ok")
